# revision 4
# baseline (speedup 1.0000x reference)
"""Bass/Trainium2 kernel for nn_DotProductAttention_47528108097846.

reference:
    scores = einsum('bhqd,bhkd->bhqk', Q, K) / 16
    attn = softmax(scores, axis=-1)
    h = einsum('bhqk,bhkd->bhqd', attn, V)
    return reshape(h, (S, B, H, D))

B=2, H=8, S=4096, D=64. 16 (b,h) pairs sharded as 2 per NeuronCore across 8
cores (batch+head parallel, no cross-core comms).

Per-core algorithm (2 heads), all matmuls bf16 (weights zero-padded to 128
rows/cols so FastWeightLoad hides the per-matmul weight reload; accumulation
is always fp32 in PSUM):
  - PE-transpose Q,K into QT,KT [64, 4096] bf16, build V' = [V | 1 | 0pad]
    [128, 128] per 128-wide k-block.
  - For each 1024-wide q-group, for each k-block kb:
      scoresT[kb] [128,1024] = (lhsT=KT_kb).T @ (rhs=QT slice)   (PSUM fp32)
      expT: columns [0:EXP_CS) on ScalarE (exp activation, scale 1/16,
            bias 16*ln(d0)); columns [EXP_CS:] on the DVE via a custom
            8-stage op computing ((c0*s + c1)*s + 1)^16 ~= exp(s/16)/d0^16
            (deg-2 fit + 4 squarings). Softmax cancels the common d0^16.
      outT [128,1024] += (lhsT=V'_kb).T @ expT   (accumulating matmul; row 64
                                           = sum of exp = softmax denominator)
  - Transpose outT in [65,128] strips to [128,65] (PE), reciprocal of col 64
    (DVE), multiply cols 0:64 by it (ScalarE activation-with-scale), DMA out.

Engine budget per core (measured baseline 334.6us was ScalarE-bound at
288us): after the exp split ScalarE ~205us, DVE ~205us, PE ~230us -> PE
becomes the roofline.
"""
import numpy as np

import concourse.bass as bass
import concourse.bacc as bacc
import concourse.tile as tile
from concourse import mybir
from concourse.masks import make_identity
from concourse.bass_utils import run_bass_kernel_spmd

B, H, S, D = 2, 8, 4096, 64
N_CORES = 8
PAIRS_PER_CORE = (B * H) // N_CORES  # 2 heads per core

f32 = mybir.dt.float32
f32r = mybir.dt.float32r
bf16 = mybir.dt.bfloat16

QG = 1024            # q-group width (psum scores buffer = QG*4B = 2 banks)
NQG = S // QG        # 4 q-groups per head
NKB = S // 128       # 32 k-blocks per head

# ---------------------------------------------------------------------------
# Custom DVE op: EXP16 -- out = ((c0*s + c1)*s + 1)^16 ~= exp(s/16)/d0^16.
# Deg-2 least-squares fit of e^u/d0 on u = s/256 in [-0.22, 0.22] (covers
# |s| <= 56; randn scores have sigma=8). ScalarE's exp gets bias 16*ln(d0)
# so both engines produce identically scaled values.
EXP16_NAME = "EXP16_POLY_ANT"
EXP_D0 = 1.0000875648796109
EXP_E1 = 1.0070340603478836
EXP_E2 = 0.49672662859727144
EXP_C0 = float(EXP_E2 / 256.0**2)
EXP_C1 = float(EXP_E1 / 256.0)
EXP_BIAS = float(16.0 * np.log(EXP_D0))

# Exp column split at the PSUM bank boundary: ScalarE owns cols [0:512)
# (bank A), the DVE custom op owns [512:1024) (bank B). Each bank then has
# an independent QK -> exp -> QK(+2) chain and each AV j-half waits on
# exactly one engine.
EXP_CS = 512


def _np_exp16(in0, in1, s0, s1, imm2):
    q = (in0.astype(np.float32) * s0 + s1) * in0 + 1.0
    q = q * q
    q = q * q
    q = q * q
    return q * q


def register_exp16():
    import concourse.dve_ops as dve_ops_mod
    from concourse.dve_ops import DveOp
    from concourse.dve_spec import C0, C1, One, Spec, Src0, lower, _has_src1
    from concourse.dve_uop import DveOpSpec

    for op in dve_ops_mod.OPS:
        if op.name == EXP16_NAME:
            return op
    m1 = Src0 * C0
    a1 = m1 + C1
    m2 = a1 * Src0
    a2 = m2 + One
    y1 = a2 * a2
    y2 = y1 * y1
    y3 = y2 * y2
    y4 = y3 * y3
    spec = Spec(body=y4, reference=_np_exp16)
    row = dve_ops_mod._CUSTOM_DVE_ROW_BASE + len(dve_ops_mod.OPS)
    assert row < 0x20, "no free custom-DVE rows"
    dve_ops_mod._SUB_OPCODE_FOR_NAME[EXP16_NAME] = row
    shas = {}
    for ver in ("v3", "v4"):
        try:
            uops = lower(spec, ver=ver)
        except Exception:
            continue
        shas[ver] = DveOpSpec(
            name=EXP16_NAME, opcode=row, uops=uops, rd1_en=_has_src1(spec)
        ).sha(ver)
    op = DveOp(EXP16_NAME, spec, subdim=False, uops_sha=shas)
    dve_ops_mod.OPS.append(op)
    dve_ops_mod.CUSTOM_DVE_SPECS[EXP16_NAME] = spec
    return op


def build_attention(nc, tc, q, k, v, o, qk_dtype=bf16, av_dtype=bf16,
                    repeat_loop=None, mode="full"):
    """Emit attention for PAIRS_PER_CORE heads.

    q/k/v/o: DRAM APs of shape [PAIRS_PER_CORE, S, D] (fp32).
    """
    import contextlib
    exp16 = register_exp16()
    ctx = contextlib.ExitStack()
    consts = ctx.enter_context(tc.tile_pool(name="consts", bufs=1))
    nat = ctx.enter_context(tc.tile_pool(name="nat", bufs=2))
    persist = ctx.enter_context(tc.tile_pool(name="persist", bufs=1))
    sb = ctx.enter_context(tc.tile_pool(name="sb", bufs=3))
    pool_e = ctx.enter_context(tc.tile_pool(name="sb_e", bufs=6))
    pool_s = ctx.enter_context(tc.tile_pool(name="ps_s", bufs=2, space="PSUM"))
    pool_o = ctx.enter_context(tc.tile_pool(name="ps_o", bufs=1, space="PSUM"))
    pool_t = ctx.enter_context(tc.tile_pool(name="ps_t", bufs=2, space="PSUM"))

    if mode == "copyonly":
        for h in range(PAIRS_PER_CORE):
            t = None
            for src in (q, k, v):
                t = nat.tile([128, NKB, 64], f32, tag="nat")
                nc.sync.dma_start(
                    out=t, in_=src[h].rearrange("(n p) d -> p n d", p=128))
            nc.sync.dma_start(
                out=o[h].rearrange("(n p) d -> p n d", p=128), in_=t)
        ctx.close()
        return

    # exp bias for ScalarE (matches the DVE poly's d0^16 scale) + act-table
    # preload: a dummy 1-element exp right at kernel start pulls the 1.3us
    # ACT_TABLE_LOAD off the first real exp's critical path
    bias_ap = consts.tile([128, 1], f32)
    nc.vector.memset(bias_ap, EXP_BIAS)
    dummy = consts.tile([128, 1], f32)
    nc.vector.memset(dummy, 0.0)
    dummy_o = consts.tile([128, 1], bf16)
    nc.scalar.activation(out=dummy_o, in_=dummy,
                         func=mybir.ActivationFunctionType.Exp,
                         bias=bias_ap, scale=1.0 / 16.0)

    ident = consts.tile([128, 128], f32)
    make_identity(nc, ident)
    identb = consts.tile([128, 128], qk_dtype)
    nc.vector.tensor_copy(out=identb, in_=ident)

    # ---------------- prologue: load + transpose Q,K; build V' ----------
    # QT/KT padded to 128 contraction rows (rows 64.. are zero) and V' padded
    # to 128 columns (cols 65.. zero) so every matmul carries full 128-wide
    # bf16 weights -> FastWeightLoad can hide the per-matmul weight reload.
    qts, kts, v1s = [], [], []

    def emit_prologue(h):
        qt = persist.tile([128, NKB, 128], qk_dtype, tag=f"qt{h}")
        kt = persist.tile([128, NKB, 128], qk_dtype, tag=f"kt{h}")
        v1 = persist.tile([128, NKB, 128], av_dtype, tag=f"v1{h}")
        qts.append(qt)
        kts.append(kt)
        v1s.append(v1)
        if h == 0:
            # parallelize the two zero-fills that gate the first QK matmul
            nc.gpsimd.memset(kt[64:128], 0.0)
            nc.vector.memset(qt[64:128], 0.0)
        else:
            nc.gpsimd.memset(qt[64:128], 0.0)
            nc.gpsimd.memset(kt[64:128], 0.0)

        # interleave K/Q chunk loads+transposes (K first) so the first QK
        # matmul and first exp can start as early as possible
        CH = 8
        for g in range(NKB // CH):
            for (src, dst) in ((k, kt), (q, qt)):
                natc = nat.tile([128, CH, 64], f32, tag="nat")
                nc.sync.dma_start(
                    out=natc,
                    in_=src[h].rearrange("(n p) d -> p n d", p=128)[
                        :, g * CH:(g + 1) * CH, :])
                natbc = nat.tile([128, CH, 64], qk_dtype, tag="natb")
                if h == 0 and g == 0:
                    nc.vector.tensor_copy(out=natbc, in_=natc)
                else:
                    nc.gpsimd.tensor_copy(out=natbc, in_=natc)
                ps_tr = pool_t.tile([64, CH, 128], qk_dtype, tag="t")
                for j in range(CH):
                    nc.tensor.transpose(ps_tr[:, j, :], natbc[:, j, :], identb)
                nc.vector.tensor_copy(
                    out=dst[0:64, g * CH:(g + 1) * CH, :], in_=ps_tr)
            if g == 2:
                # V' build deferred past the first K/Q chunks; ones column +
                # zero pad on idle GpSimd
                nc.gpsimd.memset(v1[:, :, 64:65], 1.0)
                nc.gpsimd.memset(v1[:, :, 65:128], 0.0)
                vnat = nat.tile([128, NKB, 64], f32, tag="vnat")
                nc.sync.dma_start(
                    out=vnat, in_=v[h].rearrange("(n p) d -> p n d", p=128))
                nc.gpsimd.tensor_copy(out=v1[:, :, 0:64], in_=vnat)

    # head 0 upfront; later heads' prologues are emitted inside head 0's
    # main loop (after its first q-group) so their PE-transpose bursts
    # spread out instead of starving the exp engines early on
    emit_prologue(0)
    defer_prologues = (repeat_loop is None and mode == "full")
    if not defer_prologues:
        for h in range(1, PAIRS_PER_CORE):
            emit_prologue(h)

    # ---------------- main loops --------------------------------------
    def main_compute():
        for h in range(PAIRS_PER_CORE):
            qt, kt, v1 = qts[h], kts[h], v1s[h]
            out_r = o[h].rearrange("(n p) d -> p n d", p=128)
            for qg in range(NQG):
                ps_o = pool_o.tile([128, QG], f32, tag="o")

                def av(prev_eT, prev_kb, j):
                    nc.tensor.matmul(
                        out=ps_o[:, j * 512:(j + 1) * 512],
                        lhsT=v1[:, prev_kb, :],
                        rhs=prev_eT[:, j * 512:(j + 1) * 512],
                        start=(prev_kb == 0), stop=(prev_kb == NKB - 1))

                # software-pipelined at depth 2: QK(kb) matmuls interleave
                # with the accumulating AV matmuls of kb-2, giving each
                # split exp ~2 tiles (~1.7us) of PE work to hide its
                # latency + semaphore hops behind (the PE executes its
                # queue strictly in order, so a not-yet-ready AV stalls
                # everything behind it)
                pend = []
                for kb in range(NKB):
                    ps_s = pool_s.tile([128, QG], f32, tag="s")
                    for j in range(QG // 512):
                        nc.tensor.matmul(
                            out=ps_s[:, j * 512:(j + 1) * 512],
                            lhsT=kt[:, kb, :],
                            rhs=qt.rearrange("p n d -> p (n d)")[
                                :, qg * QG + j * 512: qg * QG + (j + 1) * 512],
                            start=True, stop=True)
                        if len(pend) >= 2:
                            av(pend[0][0], pend[0][1], j)
                            if j == QG // 512 - 1:
                                pend.pop(0)
                    eT = pool_e.tile([128, QG], av_dtype, tag="exp")
                    # exp split across the two engines (softmax cancels the
                    # shared d0^16 scale)
                    nc.vector._custom_dve(
                        exp16, out=eT[:, EXP_CS:QG], in0=ps_s[:, EXP_CS:QG],
                        s0=EXP_C0, s1=EXP_C1)
                    nc.scalar.activation(
                        out=eT[:, 0:EXP_CS], in_=ps_s[:, 0:EXP_CS],
                        func=mybir.ActivationFunctionType.Exp,
                        bias=bias_ap, scale=1.0 / 16.0)
                    pend.append((eT, kb))
                for eT_p, kb_p in pend:
                    for j in range(QG // 512):
                        av(eT_p, kb_p, j)
                # epilogue for this q-group
                oT = sb.tile([65, QG], f32, tag="oT")
                nc.vector.tensor_copy(out=oT, in_=ps_o[0:65, :])
                out_sb = sb.tile([128, QG // 128, 64], f32, tag="out")
                for i in range(QG // 128):
                    ps_t = pool_t.tile([128, 65], f32, tag="t")
                    nc.tensor.transpose(
                        ps_t, oT[:, i * 128:(i + 1) * 128],
                        ident[0:65, 0:65])
                    rcp = sb.tile([128, 1], f32, tag="rcp")
                    nc.vector.reciprocal(out=rcp, in_=ps_t[:, 64:65])
                    # scale-by-reciprocal on ScalarE (activation Copy with
                    # per-partition scale) to keep the DVE free for exps
                    nc.scalar.activation(
                        out=out_sb[:, i, :], in_=ps_t[:, 0:64],
                        func=mybir.ActivationFunctionType.Copy,
                        scale=rcp)
                    if i == 3:
                        nc.sync.dma_start(
                            out=out_r[:, qg * (QG // 128):qg * (QG // 128) + 4, :],
                            in_=out_sb[:, 0:4, :])
                nc.sync.dma_start(
                    out=out_r[:, qg * (QG // 128) + 4:(qg + 1) * (QG // 128), :],
                    in_=out_sb[:, 4:8, :])
                if defer_prologues and h == 0 and qg == 0:
                    for h2 in range(1, PAIRS_PER_CORE):
                        emit_prologue(h2)

    if mode == "prologue":
        pass
    elif repeat_loop is None:
        main_compute()
    else:
        with tc.For_i(0, repeat_loop, 1) as _:
            main_compute()
    ctx.close()


_CACHED = {}


def build_program(qk_dtype=bf16, av_dtype=bf16, repeat_loop=None, mode="full"):
    key = (str(qk_dtype), str(av_dtype), repeat_loop, mode)
    if key in _CACHED:
        return _CACHED[key]
    nc = bacc.Bacc("TRN2", target_bir_lowering=False, debug=False,
                   num_devices=N_CORES)
    q = nc.dram_tensor("q", [PAIRS_PER_CORE, S, D], f32,
                       kind="ExternalInput").ap()
    k = nc.dram_tensor("k", [PAIRS_PER_CORE, S, D], f32,
                       kind="ExternalInput").ap()
    v = nc.dram_tensor("v", [PAIRS_PER_CORE, S, D], f32,
                       kind="ExternalInput").ap()
    o = nc.dram_tensor("o", [PAIRS_PER_CORE, S, D], f32,
                       kind="ExternalOutput").ap()
    with tile.TileContext(nc) as tc:
        build_attention(nc, tc, q, k, v, o, qk_dtype=qk_dtype,
                        av_dtype=av_dtype, repeat_loop=repeat_loop, mode=mode)
    nc.compile()
    _CACHED[key] = nc
    return nc


def kernel(queries, keys, values, adj=None, **_unused):
    """Full-input attention on 8 NeuronCores. Returns [S, B, H, D] fp32."""
    queries = np.ascontiguousarray(queries, dtype=np.float32)
    keys = np.ascontiguousarray(keys, dtype=np.float32)
    values = np.ascontiguousarray(values, dtype=np.float32)

    nc = build_program()
    qf = queries.reshape(B * H, S, D)
    kf = keys.reshape(B * H, S, D)
    vf = values.reshape(B * H, S, D)
    in_maps = []
    for c in range(N_CORES):
        sl = slice(c * PAIRS_PER_CORE, (c + 1) * PAIRS_PER_CORE)
        in_maps.append({"q": qf[sl], "k": kf[sl], "v": vf[sl]})
    res = run_bass_kernel_spmd(nc, in_maps, list(range(N_CORES)))
    hout = np.empty((B * H, S, D), dtype=np.float32)
    for c in range(N_CORES):
        hout[c * PAIRS_PER_CORE:(c + 1) * PAIRS_PER_CORE] = res.results[c]["o"]
    return hout.reshape(B, H, S, D).reshape(S, B, H, D)


# revision 7
# speedup vs baseline: 1.3333x; 1.3333x over previous
"""Bass/Trainium2 kernel for nn_DotProductAttention_47528108097846.

reference:
    scores = einsum('bhqd,bhkd->bhqk', Q, K) / 16
    attn = softmax(scores, axis=-1)
    h = einsum('bhqk,bhkd->bhqd', attn, V)
    return reshape(h, (S, B, H, D))

B=2, H=8, S=4096, D=64. 16 (b,h) pairs sharded as 2 per NeuronCore across 8
cores (batch+head parallel, no cross-core comms).

Per-core algorithm (2 heads), all matmuls bf16 (weights zero-padded to 128
rows/cols so FastWeightLoad hides the per-matmul weight reload; accumulation
is always fp32 in PSUM):
  - PE-transpose Q,K into QT,KT [64, 4096] bf16, build V' = [V | 1 | 0pad]
    [128, 128] per 128-wide k-block.
  - For each 1024-wide q-group, for each k-block kb:
      scoresT[kb] [128,1024] = (lhsT=KT_kb).T @ (rhs=QT slice)   (PSUM fp32)
      expT: columns [0:EXP_CS) on ScalarE (exp activation, scale 1/16,
            bias 16*ln(d0)); columns [EXP_CS:] on the DVE via a custom
            8-stage op computing ((c0*s + c1)*s + 1)^16 ~= exp(s/16)/d0^16
            (deg-2 fit + 4 squarings). Softmax cancels the common d0^16.
      outT [128,1024] += (lhsT=V'_kb).T @ expT   (accumulating matmul; row 64
                                           = sum of exp = softmax denominator)
  - Transpose outT in [65,128] strips to [128,65] (PE), reciprocal of col 64
    (DVE), multiply cols 0:64 by it (ScalarE activation-with-scale), DMA out.

Engine budget per core (measured baseline 334.6us was ScalarE-bound at
288us): after the exp split ScalarE ~205us, DVE ~205us, PE ~230us -> PE
becomes the roofline.
"""
import numpy as np

import concourse.bass as bass
import concourse.bacc as bacc
import concourse.tile as tile
from concourse import mybir
from concourse.masks import make_identity
from concourse.bass_utils import run_bass_kernel_spmd

B, H, S, D = 2, 8, 4096, 64
N_CORES = 8
PAIRS_PER_CORE = (B * H) // N_CORES  # 2 heads per core

f32 = mybir.dt.float32
f32r = mybir.dt.float32r
bf16 = mybir.dt.bfloat16

QG = 1024            # q-group width (psum scores buffer = QG*4B = 2 banks)
NQG = S // QG        # 4 q-groups per head
NKB = S // 128       # 32 k-blocks per head

# ---------------------------------------------------------------------------
# Custom DVE op: EXP16 -- out = ((c0*s + c1)*s + 1)^16 ~= exp(s/16)/d0^16.
# Deg-2 least-squares fit of e^u/d0 on u = s/256 in [-0.22, 0.22] (covers
# |s| <= 56; randn scores have sigma=8). ScalarE's exp gets bias 16*ln(d0)
# so both engines produce identically scaled values.
EXP16_NAME = "EXP16_POLY_ANT"
EXP_D0 = 1.0000875648796109
EXP_E1 = 1.0070340603478836
EXP_E2 = 0.49672662859727144
EXP_C0 = float(EXP_E2 / 256.0**2)
EXP_C1 = float(EXP_E1 / 256.0)
EXP_BIAS = float(16.0 * np.log(EXP_D0))

# Exp column split at the PSUM bank boundary: ScalarE owns cols [0:512)
# (bank A), the DVE custom op owns [512:1024) (bank B). Each bank then has
# an independent QK -> exp -> QK(+2) chain and each AV j-half waits on
# exactly one engine.
EXP_CS = 512


def _np_exp16(in0, in1, s0, s1, imm2):
    q = (in0.astype(np.float32) * s0 + s1) * in0 + 1.0
    q = q * q
    q = q * q
    q = q * q
    return q * q


def register_exp16():
    import concourse.dve_ops as dve_ops_mod
    from concourse.dve_ops import DveOp
    from concourse.dve_spec import C0, C1, One, Spec, Src0, lower, _has_src1
    from concourse.dve_uop import DveOpSpec

    for op in dve_ops_mod.OPS:
        if op.name == EXP16_NAME:
            return op
    m1 = Src0 * C0
    a1 = m1 + C1
    m2 = a1 * Src0
    a2 = m2 + One
    y1 = a2 * a2
    y2 = y1 * y1
    y3 = y2 * y2
    y4 = y3 * y3
    spec = Spec(body=y4, reference=_np_exp16)
    row = dve_ops_mod._CUSTOM_DVE_ROW_BASE + len(dve_ops_mod.OPS)
    assert row < 0x20, "no free custom-DVE rows"
    dve_ops_mod._SUB_OPCODE_FOR_NAME[EXP16_NAME] = row
    shas = {}
    for ver in ("v3", "v4"):
        try:
            uops = lower(spec, ver=ver)
        except Exception:
            continue
        shas[ver] = DveOpSpec(
            name=EXP16_NAME, opcode=row, uops=uops, rd1_en=_has_src1(spec)
        ).sha(ver)
    op = DveOp(EXP16_NAME, spec, subdim=False, uops_sha=shas)
    dve_ops_mod.OPS.append(op)
    dve_ops_mod.CUSTOM_DVE_SPECS[EXP16_NAME] = spec
    return op


def build_attention(nc, tc, q, k, v, o, qk_dtype=bf16, av_dtype=bf16,
                    repeat_loop=None, mode="full"):
    """Emit attention for PAIRS_PER_CORE heads.

    q/k/v/o: DRAM APs of shape [PAIRS_PER_CORE, S, D] (fp32).
    """
    import contextlib
    exp16 = register_exp16()
    ctx = contextlib.ExitStack()
    consts = ctx.enter_context(tc.tile_pool(name="consts", bufs=1))
    nat = ctx.enter_context(tc.tile_pool(name="nat", bufs=2))
    persist = ctx.enter_context(tc.tile_pool(name="persist", bufs=1))
    sb = ctx.enter_context(tc.tile_pool(name="sb", bufs=3))
    pool_e = ctx.enter_context(tc.tile_pool(name="sb_e", bufs=6))
    # two independent single-bank score pools: PSUM pool rotation is
    # tile-granular, so bank A's reuse must not wait on bank B's reader
    pool_sA = ctx.enter_context(tc.tile_pool(name="ps_sA", bufs=2, space="PSUM"))
    pool_sB = ctx.enter_context(tc.tile_pool(name="ps_sB", bufs=2, space="PSUM"))
    pool_o = ctx.enter_context(tc.tile_pool(name="ps_o", bufs=1, space="PSUM"))
    pool_t = ctx.enter_context(tc.tile_pool(name="ps_t", bufs=2, space="PSUM"))

    if mode == "copyonly":
        for h in range(PAIRS_PER_CORE):
            t = None
            for src in (q, k, v):
                t = nat.tile([128, NKB, 64], f32, tag="nat")
                nc.sync.dma_start(
                    out=t, in_=src[h].rearrange("(n p) d -> p n d", p=128))
            nc.sync.dma_start(
                out=o[h].rearrange("(n p) d -> p n d", p=128), in_=t)
        ctx.close()
        return

    # exp bias for ScalarE (matches the DVE poly's d0^16 scale) + act-table
    # preload: a dummy 1-element exp right at kernel start pulls the 1.3us
    # ACT_TABLE_LOAD off the first real exp's critical path
    bias_ap = consts.tile([128, 1], f32)
    nc.vector.memset(bias_ap, EXP_BIAS)
    dummy = consts.tile([128, 1], f32)
    nc.vector.memset(dummy, 0.0)
    dummy_o = consts.tile([128, 1], bf16)
    nc.scalar.activation(out=dummy_o, in_=dummy,
                         func=mybir.ActivationFunctionType.Exp,
                         bias=bias_ap, scale=1.0 / 16.0)

    ident = consts.tile([128, 128], f32)
    make_identity(nc, ident)
    identb = consts.tile([128, 128], qk_dtype)
    nc.vector.tensor_copy(out=identb, in_=ident)

    # ---------------- prologue: load + transpose Q,K; build V' ----------
    # QT/KT padded to 128 contraction rows (rows 64.. are zero) and V' padded
    # to 128 columns (cols 65.. zero) so every matmul carries full 128-wide
    # bf16 weights -> FastWeightLoad can hide the per-matmul weight reload.
    qts, kts, v1s = [], [], []

    def emit_prologue(h):
        qt = persist.tile([128, NKB, 128], qk_dtype, tag=f"qt{h}")
        kt = persist.tile([128, NKB, 128], qk_dtype, tag=f"kt{h}")
        v1 = persist.tile([128, NKB, 128], av_dtype, tag=f"v1{h}")
        qts.append(qt)
        kts.append(kt)
        v1s.append(v1)
        if h == 0:
            # parallelize the two zero-fills that gate the first QK matmul
            nc.gpsimd.memset(kt[64:128], 0.0)
            nc.vector.memset(qt[64:128], 0.0)
        else:
            nc.gpsimd.memset(qt[64:128], 0.0)
            nc.gpsimd.memset(kt[64:128], 0.0)

        # interleave K/Q chunk loads+transposes (K first) so the first QK
        # matmul and first exp can start as early as possible
        CH = 8
        for g in range(NKB // CH):
            for (src, dst) in ((k, kt), (q, qt)):
                natc = nat.tile([128, CH, 64], f32, tag="nat")
                nc.sync.dma_start(
                    out=natc,
                    in_=src[h].rearrange("(n p) d -> p n d", p=128)[
                        :, g * CH:(g + 1) * CH, :])
                natbc = nat.tile([128, CH, 64], qk_dtype, tag="natb")
                if h == 0 and g == 0:
                    nc.vector.tensor_copy(out=natbc, in_=natc)
                else:
                    nc.gpsimd.tensor_copy(out=natbc, in_=natc)
                ps_tr = pool_t.tile([64, CH, 128], qk_dtype, tag="t")
                for j in range(CH):
                    nc.tensor.transpose(ps_tr[:, j, :], natbc[:, j, :], identb)
                nc.vector.tensor_copy(
                    out=dst[0:64, g * CH:(g + 1) * CH, :], in_=ps_tr)
            if g == 2:
                # V' build deferred past the first K/Q chunks; ones column +
                # zero pad on idle GpSimd
                nc.gpsimd.memset(v1[:, :, 64:65], 1.0)
                nc.gpsimd.memset(v1[:, :, 65:128], 0.0)
                vnat = nat.tile([128, NKB, 64], f32, tag="vnat")
                nc.sync.dma_start(
                    out=vnat, in_=v[h].rearrange("(n p) d -> p n d", p=128))
                nc.gpsimd.tensor_copy(out=v1[:, :, 0:64], in_=vnat)

    # head 0 upfront; later heads' prologues are emitted inside head 0's
    # main loop (after its first q-group) so their PE-transpose bursts
    # spread out instead of starving the exp engines early on
    emit_prologue(0)
    defer_prologues = (repeat_loop is None and mode == "full")
    if not defer_prologues:
        for h in range(1, PAIRS_PER_CORE):
            emit_prologue(h)

    # ---------------- main loops --------------------------------------
    def main_compute():
        for h in range(PAIRS_PER_CORE):
            qt, kt, v1 = qts[h], kts[h], v1s[h]
            out_r = o[h].rearrange("(n p) d -> p n d", p=128)
            for qg in range(NQG):
                ps_o = pool_o.tile([128, QG], f32, tag="o")

                def av(prev_eT, prev_kb, j):
                    nc.tensor.matmul(
                        out=ps_o[:, j * 512:(j + 1) * 512],
                        lhsT=v1[:, prev_kb, :],
                        rhs=prev_eT[:, j * 512:(j + 1) * 512],
                        start=(prev_kb == 0), stop=(prev_kb == NKB - 1))

                # software-pipelined at depth 2: QK(kb) matmuls interleave
                # with the accumulating AV matmuls of kb-2, giving each
                # split exp ~2 tiles (~1.7us) of PE work to hide its
                # latency + semaphore hops behind (the PE executes its
                # queue strictly in order, so a not-yet-ready AV stalls
                # everything behind it)
                pend = []
                for kb in range(NKB):
                    ps_sa = pool_sA.tile([128, 512], f32, tag="sA")
                    ps_sb = pool_sB.tile([128, 512], f32, tag="sB")
                    ps_sj = (ps_sa, ps_sb)
                    eT = pool_e.tile([128, QG], av_dtype, tag="exp")
                    for j in range(QG // 512):
                        nc.tensor.matmul(
                            out=ps_sj[j],
                            lhsT=kt[:, kb, :],
                            rhs=qt.rearrange("p n d -> p (n d)")[
                                :, qg * QG + j * 512: qg * QG + (j + 1) * 512],
                            start=True, stop=True)
                        # exp of this half right after its producing matmul:
                        # bank A (j=0) -> ScalarE, bank B (j=1) -> DVE custom
                        # op (softmax cancels the shared d0^16 scale)
                        if j == 0:
                            nc.scalar.activation(
                                out=eT[:, 0:512], in_=ps_sj[0],
                                func=mybir.ActivationFunctionType.Exp,
                                bias=bias_ap, scale=1.0 / 16.0)
                        else:
                            nc.vector._custom_dve(
                                exp16, out=eT[:, 512:QG], in0=ps_sj[1],
                                s0=EXP_C0, s1=EXP_C1)
                        if len(pend) >= 2:
                            av(pend[0][0], pend[0][1], j)
                            if j == QG // 512 - 1:
                                pend.pop(0)
                    pend.append((eT, kb))
                for eT_p, kb_p in pend:
                    for j in range(QG // 512):
                        av(eT_p, kb_p, j)
                # epilogue for this q-group
                oT = sb.tile([65, QG], f32, tag="oT")
                nc.vector.tensor_copy(out=oT, in_=ps_o[0:65, :])
                out_sb = sb.tile([128, QG // 128, 64], f32, tag="out")
                for i in range(QG // 128):
                    ps_t = pool_t.tile([128, 65], f32, tag="t")
                    nc.tensor.transpose(
                        ps_t, oT[:, i * 128:(i + 1) * 128],
                        ident[0:65, 0:65])
                    rcp = sb.tile([128, 1], f32, tag="rcp")
                    nc.vector.reciprocal(out=rcp, in_=ps_t[:, 64:65])
                    # scale-by-reciprocal on ScalarE (activation Copy with
                    # per-partition scale) to keep the DVE free for exps
                    nc.scalar.activation(
                        out=out_sb[:, i, :], in_=ps_t[:, 0:64],
                        func=mybir.ActivationFunctionType.Copy,
                        scale=rcp)
                    if i == 3:
                        nc.sync.dma_start(
                            out=out_r[:, qg * (QG // 128):qg * (QG // 128) + 4, :],
                            in_=out_sb[:, 0:4, :])
                nc.sync.dma_start(
                    out=out_r[:, qg * (QG // 128) + 4:(qg + 1) * (QG // 128), :],
                    in_=out_sb[:, 4:8, :])
                if defer_prologues and h == 0 and qg == 0:
                    for h2 in range(1, PAIRS_PER_CORE):
                        emit_prologue(h2)

    if mode == "prologue":
        pass
    elif repeat_loop is None:
        main_compute()
    else:
        with tc.For_i(0, repeat_loop, 1) as _:
            main_compute()
    ctx.close()


_CACHED = {}


def build_program(qk_dtype=bf16, av_dtype=bf16, repeat_loop=None, mode="full"):
    key = (str(qk_dtype), str(av_dtype), repeat_loop, mode)
    if key in _CACHED:
        return _CACHED[key]
    nc = bacc.Bacc("TRN2", target_bir_lowering=False, debug=False,
                   num_devices=N_CORES)
    q = nc.dram_tensor("q", [PAIRS_PER_CORE, S, D], f32,
                       kind="ExternalInput").ap()
    k = nc.dram_tensor("k", [PAIRS_PER_CORE, S, D], f32,
                       kind="ExternalInput").ap()
    v = nc.dram_tensor("v", [PAIRS_PER_CORE, S, D], f32,
                       kind="ExternalInput").ap()
    o = nc.dram_tensor("o", [PAIRS_PER_CORE, S, D], f32,
                       kind="ExternalOutput").ap()
    with tile.TileContext(nc) as tc:
        build_attention(nc, tc, q, k, v, o, qk_dtype=qk_dtype,
                        av_dtype=av_dtype, repeat_loop=repeat_loop, mode=mode)
    nc.compile()
    _CACHED[key] = nc
    return nc


def kernel(queries, keys, values, adj=None, **_unused):
    """Full-input attention on 8 NeuronCores. Returns [S, B, H, D] fp32."""
    queries = np.ascontiguousarray(queries, dtype=np.float32)
    keys = np.ascontiguousarray(keys, dtype=np.float32)
    values = np.ascontiguousarray(values, dtype=np.float32)

    nc = build_program()
    qf = queries.reshape(B * H, S, D)
    kf = keys.reshape(B * H, S, D)
    vf = values.reshape(B * H, S, D)
    in_maps = []
    for c in range(N_CORES):
        sl = slice(c * PAIRS_PER_CORE, (c + 1) * PAIRS_PER_CORE)
        in_maps.append({"q": qf[sl], "k": kf[sl], "v": vf[sl]})
    res = run_bass_kernel_spmd(nc, in_maps, list(range(N_CORES)))
    hout = np.empty((B * H, S, D), dtype=np.float32)
    for c in range(N_CORES):
        hout[c * PAIRS_PER_CORE:(c + 1) * PAIRS_PER_CORE] = res.results[c]["o"]
    return hout.reshape(B, H, S, D).reshape(S, B, H, D)
